# revision 36
# baseline (speedup 1.0000x reference)
"""Trainium2 Bass kernel for multi-head self-attention (B=2, N=2048, C=1024, H=16, d=64).

Sharding: 8 cores = 2 batches x 4 head-groups (4 heads each). Each core computes
QKV for its heads (column-sliced W_qkv), full attention over its heads, and a
row-sliced partial of the output projection. Host sums the 4 partials per batch
and adds b_proj.

Device dataflow (per core, all matmuls bf16 with fp32 PSUM accumulation):
  - x^T is loaded [C, N] so Q^T/K^T come out as [head*d, N] (d on partitions),
    which is exactly the lhsT/rhs layout the scores matmul wants.
  - S^T tile [128 keys, 512 queries] = (K^T chunk)^T-matmul(Q^T chunk), K=64
    contraction; the two heads of a pair sit at partition offsets 0/64.
  - softmax skips the max-subtraction (scores are ~N(0,1); exp is safe in fp32)
    so exp(scale*S) is a single ACT pass straight out of PSUM, cast to bf16.
  - AV runs in O[query, d] orientation: lhsT = P^T slice [128 keys, 128
    queries], rhs = V[128 keys, 65] (65th column ones), so each matmul uses all
    128 output partitions and only costs 65 output rows - half the PE time of
    the O^T orientation.  Column 64 of the accumulator is the softmax
    denominator for each query, landing on that query's partition.
  - normalization is per-partition: one reciprocal [128, 4] per head, then 8
    tensor_scalar multiplies [128, 64] into bf16 SBUF - no broadcast matmuls.
  - O[q, 2*64] is flipped back to O^T pair-layout [128, q] with a single PE
    identity-transpose per 128-query tile, then one DVE copy into oT.
  - projection: out[q,c] = sum_p O^T-pair-chunk^T @ W_proj rows, fp32 out via
    DMA; emission deferred into later groups to keep the pipeline fed.
  - scheduling: AV matmuls lag S/exp by a few groups (software pipeline),
    producer chains drip-feed between attention groups, block order runs all
    pair-0 chunks before pair-1, and the PE is HAM-warmed during initial DMAs.
"""

import sys

sys.path.insert(0, "/opt/trn_rl_repo")

import numpy as np
import ml_dtypes

import concourse.bass as bass
import concourse.tile as tile
from concourse import bacc, masks, mybir
from concourse.bass_utils import run_bass_kernel_spmd

BF16 = ml_dtypes.bfloat16
F32 = mybir.dt.float32
BF = mybir.dt.bfloat16
AF = mybir.ActivationFunctionType

B, NT, C, H, D = 2, 2048, 1024, 16, 64
NCORES = 8
HPC = 4  # heads per core
DQ = HPC * D  # 256 c_out per q/k/v slice
VW = HPC * (D + 1)  # 260: V with a ones column per head
SCALE = D ** -0.5


def build_program(nt=NT):
    """Build the SPMD Bass program. nt parametrized so a small version can be
    simulated quickly in CoreSim."""
    n_tc = nt // 512  # 512-token chunks
    n_kt = nt // 128  # 128-key tiles
    n_ktg = nt // 256  # groups of 2 key tiles (one exp per head per 1024 cols)

    nc = bacc.Bacc("TRN2", target_bir_lowering=False, debug=False,
                   num_devices=NCORES)

    xT = nc.dram_tensor("xT", [C, nt], BF, kind="ExternalInput").ap()
    wq = nc.dram_tensor("wq", [C, DQ], BF, kind="ExternalInput").ap()
    wk = nc.dram_tensor("wk", [C, DQ], BF, kind="ExternalInput").ap()
    wv = nc.dram_tensor("wv", [C, DQ], BF, kind="ExternalInput").ap()
    wp = nc.dram_tensor("wp", [DQ, C], BF, kind="ExternalInput").ap()
    bqk = nc.dram_tensor("bqk", [128, 4], F32, kind="ExternalInput").ap()
    bvp = nc.dram_tensor("bvp", [128, 2], F32, kind="ExternalInput").ap()
    # bf16 output: halves the store-DMA time; the host accumulates partials
    # in f32.  Adds ~0.3% rounding per partial, well inside the error budget.
    out = nc.dram_tensor("out_p", [nt, C], BF, kind="ExternalOutput").ap()

    with tile.TileContext(nc) as tc:
        with (
            tc.tile_pool(name="persist", bufs=1) as persist,
            tc.tile_pool(name="pt_pool", bufs=16) as pt_pool,
            tc.tile_pool(name="stage", bufs=4) as stage,
            tc.tile_pool(name="osb_pool", bufs=2) as osb_pool,
            tc.tile_pool(name="rc_pool", bufs=2) as rc_pool,
            tc.tile_pool(name="ps_qkv", bufs=2, space="PSUM") as ps_qkv,
            tc.tile_pool(name="ps_s", bufs=2, space="PSUM") as ps_s,
            tc.tile_pool(name="ps_o", bufs=1, space="PSUM") as ps_o,
        ):
            # ---------------- persistent SBUF state ----------------
            # load order matters: wk + xT feed the first K^T chains; wv/wp
            # are only needed once attention is underway.
            xT_sb = persist.tile([128, 8, nt], BF)
            wq_sb = persist.tile([128, 8, DQ], BF)
            wk_sb = persist.tile([128, 8, DQ], BF)
            wv_sb = persist.tile([128, 8, DQ], BF)
            bqk_sb = persist.tile([128, 4], F32)
            bvp_sb = persist.tile([128, 2], F32)
            wp_sb = persist.tile([128, 2, C], BF)
            # Few big DMA instructions (the ~1.3us sequencer issue cost per
            # DMA dominates; transfers run on 16 parallel DMA engines).
            # x^T rides the SP hardware queue in 512-token chunks so the first
            # K/Q chains start early; weights ride the idle Pool (SWDGE) queue.
            xT3 = xT.rearrange("(po pi) n -> pi po n", pi=128)
            wk3 = wk.rearrange("(po pi) c -> pi po c", pi=128)
            wq3 = wq.rearrange("(po pi) c -> pi po c", pi=128)
            wv3 = wv.rearrange("(po pi) c -> pi po c", pi=128)
            wp3 = wp.rearrange("(po pi) c -> pi po c", pi=128)
            def _xt(t):
                if t == 0:
                    return
                sl = slice(t * 512, (t + 1) * 512)
                nc.sync.dma_start(xT_sb[:, 0:4, sl], xT3[:, 0:4, sl])
                nc.sync.dma_start(xT_sb[:, 4:8, sl], xT3[:, 4:8, sl])

            ones_sb = persist.tile([1, 512], BF)
            nc.vector.memset(ones_sb[:], 1.0)
            # warm the PE clock (HAM) immediately: a few tiny matmuls up
            # front get the clock ramp going while the first DMAs land
            warm_ps = ps_qkv.tile([128, 512], F32, tag="qkv", name="warm_ps")
            for i in range(8):
                nc.tensor.matmul(warm_ps[:, 0:256], ones_sb[:, 0:128],
                                 ones_sb[:, 0:256], start=(i == 0),
                                 stop=(i == 7), skip_group_check=True)
            warm_sink = persist.tile([1, 8], F32)
            nc.vector.tensor_copy(warm_sink[:, :], warm_ps[0:1, 0:8])

            # first weight/token chunks split fine so the first K-chain's
            # matmuls start as soon as contraction-chunks land
            nc.sync.dma_start(wk_sb[:, 0:2, :], wk3[:, 0:2, :])
            nc.sync.dma_start(xT_sb[:, 0:2, 0:512], xT3[:, 0:2, 0:512])
            nc.sync.dma_start(wk_sb[:, 2:4, :], wk3[:, 2:4, :])
            nc.sync.dma_start(xT_sb[:, 2:4, 0:512], xT3[:, 2:4, 0:512])
            nc.sync.dma_start(wq_sb[:, 0:4, :], wq3[:, 0:4, :])
            nc.sync.dma_start(wk_sb[:, 4:8, :], wk3[:, 4:8, :])
            nc.sync.dma_start(xT_sb[:, 4:8, 0:512], xT3[:, 4:8, 0:512])
            nc.sync.dma_start(wq_sb[:, 4:8, :], wq3[:, 4:8, :])
            nc.sync.dma_start(bqk_sb[:], bqk)
            for t in range(1, n_tc):
                _xt(t)
                if t == 1:
                    nc.sync.dma_start(bvp_sb[:], bvp)
                    nc.sync.dma_start(wv_sb[:], wv3)
                elif t == 2:
                    nc.sync.dma_start(wp_sb[:], wp3)
            if n_tc < 3:
                nc.sync.dma_start(bvp_sb[:], bvp)
                nc.sync.dma_start(wv_sb[:], wv3)
                nc.sync.dma_start(wp_sb[:], wp3)
            ident_sb = persist.tile([128, 128], BF)
            masks.make_identity(nc, ident_sb[:])

            qT_sb = [persist.tile([128, nt], BF, tag=f"qT{p}", name=f"qT{p}")
                     for p in range(2)]
            kT_sb = [persist.tile([128, nt], BF, tag=f"kT{p}", name=f"kT{p}")
                     for p in range(2)]
            oT_sb = [persist.tile([128, n_tc, 4, 128], BF, tag=f"oT{p}",
                                  name=f"oT{p}")
                     for p in range(2)]
            # V layout [keys, kt, head, 65]: col 64 of each head is a ones
            # column (preset once) so AV accumulates softmax denominators.
            v_sb = persist.tile([128, n_kt, HPC, D + 1], BF)
            nc.vector.memset(v_sb[:, :, :, 64:65], 1.0)

            # ---------------- QKV chain emitters ----------------
            def qk_chain(w_sb, bcol, dst, p, t):
                ps = ps_qkv.tile([128, 512], F32, tag="qkv")
                for ci in range(8):
                    nc.tensor.matmul(
                        ps[:, :],
                        w_sb[:, ci, p * 128:(p + 1) * 128],
                        xT_sb[:, ci, t * 512:(t + 1) * 512],
                        start=(ci == 0), stop=(ci == 7))
                nc.vector.tensor_scalar_add(dst[:, t * 512:(t + 1) * 512],
                                            ps[:, :], bqk_sb[:, bcol:bcol + 1])

            def v_chain(tt):
                ps = ps_qkv.tile([128, 8, 64], F32, tag="qkv")
                for ci in range(8):
                    nc.tensor.matmul(
                        ps[:, 0:4, :],
                        xT_sb[:, ci, tt * 128:(tt + 1) * 128],
                        wv_sb[:, ci, :],
                        start=(ci == 0), stop=(ci == 7))
                # v bias is added after normalization (it commutes through
                # the softmax average), via the oT copy's per-partition add
                nc.vector.tensor_copy(v_sb[:, tt, :, 0:64], ps[:, 0:4, :])

            # bqk_sb columns: 0,1 = q bias pair 0/1; 2,3 = k bias pair 0/1
            def k_chain(p, t):
                qk_chain(wk_sb, 2 + p, kT_sb[p], p, t)

            def q_chain(p, t):
                qk_chain(wq_sb, 0 + p, qT_sb[p], p, t)

            # Preamble: just enough for the first attention group to start.
            k_chain(0, 0)
            q_chain(0, 0)

            # Remaining producer chains, drip-fed between attention groups.
            thunks = []
            for k in range(n_ktg):
                thunks.append(lambda tt=2 * k: v_chain(tt))
                thunks.append(lambda tt=2 * k + 1: v_chain(tt))
            kq = []
            for t in range(1, n_tc):
                kq.append(lambda t=t: k_chain(0, t))
            for t in range(1, n_tc):
                kq.append(lambda t=t: q_chain(0, t))
                kq.append(lambda t=t: k_chain(1, t - 1))
            kq.append(lambda: q_chain(1, 0))
            kq.append(lambda: k_chain(1, n_tc - 1))
            for t in range(1, n_tc):
                kq.append(lambda t=t: q_chain(1, t))

            # ---------------- attention + projection ----------------
            def make_proj(qt, nh, use_act=False, pool=None):
                # use_act: at the kernel tail ACT is idle, so route the
                # PSUM->SBUF copy there and keep the DVE free
                def proj():
                    pps = (pool or ps_qkv).tile(
                        [128, 512], F32,
                        tag="s" if pool is not None else "qkv", name="pps")
                    for pp in range(2):
                        nc.tensor.matmul(
                            pps[:, :],
                            oT_sb[pp][:, qt // 4, qt % 4, :],
                            wp_sb[:, pp, nh * 512:(nh + 1) * 512],
                            start=(pp == 0), stop=(pp == 1))
                    ost = stage.tile([128, 512], BF, tag="ost", name="ost")
                    if use_act:
                        nc.scalar.copy(ost[:, :], pps[:, :])
                    else:
                        nc.vector.tensor_copy(ost[:, :], pps[:, :])
                    nc.sync.dma_start(
                        out[qt * 128:(qt + 1) * 128, nh * 512:(nh + 1) * 512],
                        ost[:, :])
                return proj

            def make_norm_dve(o_ps, rc_sb, O_sb):
                # normalization multiplies; emitted at the START of the next
                # block so the DVE has them done well before the transposes.
                def norm_dve():
                    for qi in range(4):
                        for hh in range(2):
                            nc.vector.tensor_scalar_mul(
                                O_sb[:, qi, hh, :],
                                o_ps[hh][:, qi, 0:64],
                                rc_sb[:, hh, qi, :])
                return norm_dve

            def make_norm_pe(O_sb, p, qc, last_block):
                # transpose O back to O^T pair layout + oT copy (with the
                # deferred v-bias add); emitted 2 groups into the next block.
                def norm_pe():
                    tr = ps_qkv.tile([128, 4, 128], BF, tag="qkv",
                                     name="tr_ps")
                    for qi in range(4):
                        nc.tensor.transpose(tr[:, qi, :], O_sb[:, qi, :, :],
                                            ident_sb[:, :])
                    nc.vector.tensor_scalar_add(oT_sb[p][:, qc, :, :],
                                                tr[:, :, :],
                                                bvp_sb[:, p:p + 1])
                    # queue this chunk's projection now that oT is written
                    if p == 1 and not last_block:
                        for qt4 in range(4):
                            for nh in range(2):
                                deferred.append(make_proj(qc * 4 + qt4, nh))
                return norm_pe

            J0LAG = min(3, n_ktg - 1)
            J1LAG = min(4, n_ktg - 1)
            deferred = []
            blocks = [(qc, 0) for qc in range(n_tc)] + \
                     [(qc, 1) for qc in range(n_tc)]
            npop = 1 if n_tc >= 4 else 2
            prev_norm_dve = None
            prev_norm_pe = None
            for bi, (qc, p) in enumerate(blocks):
                    o_ps = [ps_o.tile([128, 4, 65], F32, tag=f"o{_h}",
                                      name=f"o_ps{_h}")
                            for _h in range(2)]
                    # AV accumulation groups for the 4 query sub-tiles share a
                    # PSUM bank; a start=True matmul resets the whole bank, so
                    # zero the bank once and accumulate with start=False.
                    # Emitted after prev_norm's reads for non-first blocks
                    # (in-order DVE queue would deadlock otherwise).
                    def o_memset(o_ps=o_ps):
                        for _h in range(2):
                            nc.vector.memset(o_ps[_h][:, :, :], 0.0)
                    first_block = (bi == 0)
                    if first_block:
                        o_memset()
                    j0q, j1q = [], []
                    for ktg in range(n_ktg):
                        # K/Q chains may feed this very group's S matmuls, so
                        # they are emitted (higher scheduler priority) before
                        # them.  Pacing: front-load each block's chains into
                        # the early groups (no AV work there yet), and spread
                        # the inventory so the pair-1 blocks aren't starved.
                        if n_tc >= 4:
                            if bi == 0:
                                pop_here = ktg in (0, 1, 2)
                            elif bi < 4:
                                pop_here = ktg in (0, 1)
                            else:
                                pop_here = ktg in (0, 1, 2, 3)
                        else:
                            pop_here = True
                        if pop_here and kq:
                            kq.pop(0)()
                        if ktg == 0 and prev_norm_dve is not None:
                            prev_norm_dve()
                            prev_norm_dve = None
                        s_ps = [ps_s.tile([128, 1024], F32, tag="s",
                                             name=f"s_ps{_h}")
                                for _h in range(2)]
                        pt = [pt_pool.tile([128, 1024], BF, tag="pt",
                                           name=f"pt{_h}")
                              for _h in range(2)]
                        # hh-major score order: each head's exp is emitted as
                        # soon as that head's two key tiles are scored, so the
                        # ACT starts ~0.4us earlier per group
                        for hh in range(2):
                            for j in range(2):
                                kt = ktg * 2 + j
                                nc.tensor.matmul(
                                    s_ps[hh][:, j * 512:(j + 1) * 512],
                                    kT_sb[p][hh * 64:(hh + 1) * 64,
                                             kt * 128:(kt + 1) * 128],
                                    qT_sb[p][hh * 64:(hh + 1) * 64,
                                             qc * 512:(qc + 1) * 512],
                                    start=True, stop=True)
                            nc.scalar.activation(pt[hh][:, :], s_ps[hh][:, :],
                                                 AF.Exp, scale=SCALE)
                        if ktg == min(2, n_ktg - 1) and not first_block:
                            if prev_norm_pe is not None:
                                prev_norm_pe()
                                prev_norm_pe = None
                            o_memset()
                        else:
                            for _ in range(2 if len(deferred) > 1 else 1):
                                if deferred:
                                    deferred.pop(0)()

                        def make_av_half(ktg, pt, j):
                            # one key tile's 8 AV matmuls; the two halves of
                            # a ktg are emitted in different groups so the
                            # re-write of each o_ps region lands well after
                            # the previous write's semaphore.
                            def av():
                                kt = ktg * 2 + j
                                for hh in range(2):
                                    h = 2 * p + hh
                                    for qi in range(4):
                                        nc.tensor.matmul(
                                            o_ps[hh][:, qi, :],
                                            pt[hh][:, j * 512 + qi * 128:
                                                   j * 512 + (qi + 1) * 128],
                                            v_sb[:, kt, h, :],
                                            start=False,
                                            stop=(kt == 2 * n_ktg - 1),
                                            skip_group_check=True)
                            return av

                        j0q.append(make_av_half(ktg, pt, 0))
                        j1q.append(make_av_half(ktg, pt, 1))
                        if len(j1q) > J1LAG:
                            j1q.pop(0)()
                        if len(j0q) > J0LAG:
                            j0q.pop(0)()
                        if first_block:
                            # V chains feed only the (lagged) AV matmuls, so
                            # they sit at the bottom of each group where the
                            # scheduler treats them as filler
                            for _ in range(2):
                                if thunks:
                                    thunks.pop(0)()
                    while j0q or j1q:
                        if j1q:
                            j1q.pop(0)()
                        if j0q:
                            j0q.pop(0)()
                    # start of normalization: per-query denominators live in
                    # column 64 of each o_ps region; reciprocal them now
                    rc_sb = rc_pool.tile([128, 2, 4, 1], F32, tag="rc",
                                         name="rc_sb")
                    O_sb = osb_pool.tile([128, 4, 2, 64], BF, tag="osb",
                                         name="O_sb")
                    for hh in range(2):
                        nc.vector.reciprocal(rc_sb[:, hh, :, :],
                                             o_ps[hh][:, :, 64:65])
                    if bi == len(blocks) - 1:
                        tail_state = (o_ps, rc_sb, O_sb, qc)
                        prev_norm_dve = None
                        prev_norm_pe = None
                    else:
                        prev_norm_dve = make_norm_dve(o_ps, rc_sb, O_sb)
                        prev_norm_pe = make_norm_pe(O_sb, p, qc,
                                                    last_block=False)
            # tail: last block's normalization; scales prefetched onto the
            # DVE while the PE chews leftover deferred projections, then a
            # hand-interleaved transpose/copy/projection pipeline.
            lb_o_ps, lb_rc, lb_O, lb_qc = tail_state
            if deferred:
                deferred.pop(0)()
            for qi in range(4):
                for hh in range(2):
                    nc.vector.tensor_scalar_mul(
                        lb_O[:, qi, hh, :],
                        lb_o_ps[hh][:, qi, 0:64],
                        lb_rc[:, hh, qi, :])
            while deferred:
                deferred.pop(0)()
            tr_t = ps_qkv.tile([128, 4, 128], BF, tag="qkv", name="tr_ps_t")

            def tail_tr(qi):
                nc.tensor.matmul(tr_t[:, qi, :], lb_O[:, qi, :, :],
                                 ident_sb[:, :], is_transpose=True,
                                 skip_group_check=True)
                nc.vector.tensor_scalar_add(oT_sb[1][:, lb_qc, qi, :],
                                            tr_t[:, qi, :],
                                            bvp_sb[:, 1:2])

            tail_tr(0)
            tail_tr(1)
            for qi in range(4):
                qt = lb_qc * 4 + qi
                if qi + 2 < 4:
                    tail_tr(qi + 2)
                for nh in range(2):
                    # alternate the PSUM->SBUF copies between ACT and DVE so
                    # the final drain runs on two engines in parallel
                    make_proj(qt, nh, use_act=(nh == 0),
                              pool=ps_s if nh == 1 else None)()
            assert not thunks and not kq, "producer chains never emitted"

    nc.finalize()
    return nc


def make_core_inputs(x, W_qkv, b_qkv, W_proj, nt=NT):
    """Host-side shard prep: returns in_maps list for the 8 cores."""
    in_maps = []
    for core in range(NCORES):
        b, g = divmod(core, NCORES // B)
        lo, hi = g * DQ, (g + 1) * DQ
        xTb = np.ascontiguousarray(x[b].T).astype(BF16)
        wq_c = np.ascontiguousarray(W_qkv[:, lo:hi]).astype(BF16)
        wk_c = np.ascontiguousarray(W_qkv[:, C + lo:C + hi]).astype(BF16)
        wv_c = np.ascontiguousarray(W_qkv[:, 2 * C + lo:2 * C + hi]).astype(BF16)
        bvp_c = np.stack([
            b_qkv[2 * C + lo:2 * C + lo + 128],
            b_qkv[2 * C + lo + 128:2 * C + hi],
        ], axis=1).astype(np.float32)
        wp_c = np.ascontiguousarray(W_proj[lo:hi, :]).astype(BF16)
        bqk_c = np.stack([
            b_qkv[lo:lo + 128], b_qkv[lo + 128:hi],
            b_qkv[C + lo:C + lo + 128], b_qkv[C + lo + 128:C + hi],
        ], axis=1).astype(np.float32)
        in_maps.append({
            "xT": xTb[:, :nt].copy(), "wq": wq_c, "wk": wk_c, "wv": wv_c,
            "wp": wp_c, "bqk": bqk_c, "bvp": bvp_c,
        })
    return in_maps


_prog_cache = {}


def _get_program(nt=NT):
    if nt not in _prog_cache:
        _prog_cache[nt] = build_program(nt)
    return _prog_cache[nt]


def kernel(x, W_qkv, b_qkv, W_proj, b_proj, _run_kwargs=None):
    x = np.asarray(x, dtype=np.float32)
    W_qkv = np.asarray(W_qkv, dtype=np.float32)
    b_qkv = np.asarray(b_qkv, dtype=np.float32)
    W_proj = np.asarray(W_proj, dtype=np.float32)
    b_proj = np.asarray(b_proj, dtype=np.float32)

    nc = _get_program()
    in_maps = make_core_inputs(x, W_qkv, b_qkv, W_proj)
    for attempt in range(3):
        res = run_bass_kernel_spmd(nc, in_maps, core_ids=list(range(NCORES)),
                                   **(_run_kwargs or {}))
        out = np.zeros((B, NT, C), dtype=np.float32)
        for core in range(NCORES):
            b = core // (NCORES // B)
            out[b] += np.asarray(res.results[core]["out_p"],
                                 dtype=np.float32)
        if np.isfinite(out).all():
            break
        # transient device flake (observed rarely under axon): retry
    out += b_proj[None, None, :]
    if _run_kwargs:
        kernel.last_results = res
    return out
